# revision 3
# baseline (speedup 1.0000x reference)
"""Trainium2 Bass kernel for nn_Conv2d_77489799955262.

Forward value of the reference:
    y = conv2d(x, (w_pos > 0) - (w_neg > 0))      # ternary weights in {-1, 0, 1}
(the straight-through-estimator terms cancel numerically), NCHW, 3x3, stride 1,
pad 1, x [32, 256, 56, 56] f32, w [256, 256, 3, 3].

Strategy: data-parallel over batch across 8 cores (4 images per core), then
1D Winograd F(2,3) along W inside each core:

    v0 = d[2j]-d[2j+2], v1 = d[2j+1]+d[2j+2],
    v2 = d[2j+2]-d[2j+1], v3 = d[2j+1]-d[2j+3]
    m_q = sum_{kh,ci} wt[q,kh]^T v_q[row+kh]          (PE, PSUM accumulate)
    y_even = m0+m1+m2,  y_odd = m1-m2-m3              (DVE)

Engine budget per image per core (steady state):
  PE   192 MMs x 392 free dim           ~31.4 us  <- pacing engine
  DVE  8 v-ops (bf16 2x) + 32 drains    ~26 us
  ACT  8 deint copies + 8 psum copies   ~13 us
  DMA  x-in on ACT HWDGE queue, y-out on sync queue (separate FIFOs so the
       image n+1 x load never queues behind image n's y stores)

The deinterleave trick: ACT stages 4 planes per ci block in bf16,
  A[j]=d[2j], B[j]=d[2j+1], C[j]=d[2j+2], D[j]=d[2j+3]   ([56 rows x 28 j])
so every v op is a step-1, 4B-aligned bf16 tensor_tensor (v0=A-C, v1=B+C,
v2=C-B, v3=B-D) and runs in DVE 2x_1P mode (2 elem/cyc) instead of the 1x
fallback that strided fp32 reads forced (14.3 -> 7.8 us/img).
"""
import numpy as np
import ml_dtypes

import concourse.bass as bass
import concourse.tile as tile
from concourse import bacc, mybir
from concourse.bass_utils import run_bass_kernel_spmd

MODE = "wino"           # kept for test.py compatibility
PIPELINE = "slots"

N_CORES = 8
B, CI, CO, H, W, K = 32, 256, 256, 56, 56, 3
NI = B // N_CORES          # images per core
CIB = CI // 128            # ci blocks
COB = CO // 128            # co blocks
NJ = W // 2                # 28 winograd tiles per row
RB = 14                    # output rows per psum tile
NBLK = H // RB             # 4 row blocks
NMM = RB * NJ              # 392 matmul free dim
VROWS = H + 2              # 58 v rows (0 and 57 are zero pads)
PLANE = H * NJ             # 56*28 deint plane elems

F32 = mybir.dt.float32
BF16 = mybir.dt.bfloat16

_COMPILED = {}

# v_q = lhs op rhs over full [56,28] planes; letters 0..3 = A,B,C,D
_VDEF = [(0, 2, "sub"), (1, 2, "add"), (2, 1, "sub"), (1, 3, "sub")]

# deint plane defs: (letter, out col slice, x col slice (stride 2))
_DEINT = [
    (0, (1, 28), (1, 54)),    # A[j]=x[2j-1], j=1..27; A[0]=0 (pad)
    (1, (0, 28), (0, 55)),    # B[j]=x[2j]
    (2, (0, 28), (1, 56)),    # C[j]=x[2j+1]
    (3, (0, 27), (2, 55)),    # D[j]=x[2j+2], j=0..26; D[27]=0 (pad)
]

# per-group slot schedule for next-image prep (group g = co*NBLK + blk).
# deint ops in order (A0,C0,A1,C1,B0,B1,D0,D1) -> slots; v ops in order
# (q0c0,q0c1,q1c0,q1c1,q2c0,q2c1,q3c0,q3c1) -> slots.
_DEINT_SLOT = [0, 2, 2, 2, 2, 0, 0, 0]
_V_SLOT = [0, 0, 0, 1, 2, 2, 2, 1]
_DEINT_ORDER = [(0, 0), (0, 2), (1, 0), (1, 2),
                (0, 1), (1, 1), (0, 3), (1, 3)]   # (ci, letter)
_V_ORDER = [(0, 0), (1, 0), (0, 1), (1, 1),
            (0, 2), (1, 2), (0, 3), (1, 3)]       # (ci, q)


def _build(mode, iters=1, loop=0):
    nc = bacc.Bacc("TRN2", target_bir_lowering=False, debug=False,
                   num_devices=N_CORES)

    x_dram = nc.dram_tensor("x", [NI, CI, H, W], F32, kind="ExternalInput")
    w_dram = nc.dram_tensor("w", [CI, 4, 3, CO], BF16, kind="ExternalInput")
    y_dram = nc.dram_tensor("y", [NI, CO, H, W], F32, kind="ExternalOutput")

    with tile.TileContext(nc) as tc:
        with (
            tc.tile_pool(name="const", bufs=1) as cpool,
            tc.tile_pool(name="xst", bufs=2) as xpool,
            tc.tile_pool(name="abcd", bufs=2) as apool,
            tc.tile_pool(name="vst", bufs=2) as vpool,
            tc.tile_pool(name="drain", bufs=4) as dpool,
            tc.tile_pool(name="outp", bufs=4) as opool,
            tc.tile_pool(name="psum", bufs=8, space="PSUM") as ppool,
        ):
            # weights: [128ci, cib, q, kh, co]; first-needed slice on the
            # sync queue ahead of y traffic, the rest via SWDGE on gpsimd
            w_sb = cpool.tile([128, CIB, 4, 3, CO], BF16, tag="w")

            def w_slice(ci, co):
                return (w_sb[:, ci, :, :, co * 128:(co + 1) * 128],
                        w_dram[ci * 128:(ci + 1) * 128, :, :,
                               co * 128:(co + 1) * 128])

            nc.sync.dma_start(*w_slice(0, 0))
            for ci, co in ((1, 0), (0, 1), (1, 1)):
                nc.gpsimd.dma_start(*w_slice(ci, co))

            def stage_x(it, n, engs):
                """Raw (unpadded) x load, one DMA per ci block on the given
                HWDGE queues. Boundary zeros are handled in the deint
                planes, so no memsets here."""
                xt = xpool.tile([128, CIB, H * W], F32, tag="x",
                                name=f"x_{it}_{n}")
                for ci in range(CIB):
                    engs[ci].dma_start(
                        xt[:, ci, :].rearrange("p (r c) -> p r c", c=W),
                        x_dram[n, ci * 128:(ci + 1) * 128, :, :])
                return xt

            def new_abcd(it, n):
                """bf16 deint planes [ci, letter, 56*28]; zero the two pad
                columns (A col 0, D col 27)."""
                ab = apool.tile([128, CIB, 4, PLANE], BF16, tag="ab",
                                name=f"ab_{it}_{n}")
                for ci in range(CIB):
                    av = ab[:, ci, 0, :].rearrange("p (r j) -> p r j", j=NJ)
                    dv = ab[:, ci, 3, :].rearrange("p (r j) -> p r j", j=NJ)
                    nc.gpsimd.memset(av[:, :, 0:1], 0.0)
                    nc.gpsimd.memset(dv[:, :, 27:28], 0.0)
                return ab

            def new_vt(it, n):
                vt = vpool.tile([128, CIB, 4, VROWS * NJ], BF16,
                                tag="v", name=f"v_{it}_{n}")
                vv = vt[:].rearrange("p c q (r j) -> p (c q) r j", j=NJ)
                nc.gpsimd.memset(vv[:, :, 0:1, :], 0.0)
                nc.gpsimd.memset(vv[:, :, 57:58, :], 0.0)
                return vt

            def deint_op(xt, ab, ci, letter):
                """One ACT copy: fp32 strided x row -> bf16 packed plane."""
                _, (o0, o1), (i0, i1) = _DEINT[letter]
                xr = xt[:, ci, :].rearrange("p (r c) -> p r c", c=W)
                pv = (ab[:, ci, letter, :]
                      .rearrange("p (r j) -> p r j", j=NJ))
                nc.scalar.copy(pv[:, :, o0:o1], xr[:, :, i0:i1:2])

            def v_tt(ab, vt, ci, q):
                a, b, op = _VDEF[q]
                out = (vt[:, ci, q, NJ:NJ + H * NJ]
                       .rearrange("p (r j) -> p r j", j=NJ))
                la = ab[:, ci, a, :].rearrange("p (r j) -> p r j", j=NJ)
                lb = ab[:, ci, b, :].rearrange("p (r j) -> p r j", j=NJ)
                f = nc.vector.tensor_add if op == "add" else \
                    nc.vector.tensor_sub
                f(out, la, lb)

            def emit_iter(it):
                for n in range(NI):
                    if n == 0:
                        # prologue: stage + deint + v burst, dribbling in
                        # dependency order (ACT: A0,C0,A1,C1,B0,B1,D0,D1)
                        xt = stage_x(it, 0, (nc.scalar, nc.sync))
                        ab = new_abcd(it, 0)
                        vt = new_vt(it, 0)
                        for ci, letter in _DEINT_ORDER:
                            deint_op(xt, ab, ci, letter)
                        for ci, q in _V_ORDER:
                            v_tt(ab, vt, ci, q)
                        st = {"vt": vt}
                    nxt = None
                    if n + 1 < NI:
                        nxt = {}
                    emit_image(it, n, st["vt"], nxt)
                    if nxt is not None:
                        st["vt"] = nxt["vt"]

            def emit_image(it, n, vt, nxt):
                di = vi = 0
                for co in range(COB):
                    for blk in range(NBLK):
                        g = co * NBLK + blk
                        r0 = blk * RB
                        pss = [ppool.tile([128, NMM], F32, tag="ps",
                                          name=f"ps_{it}_{n}_{co}_{blk}_{q}")
                               for q in range(4)]
                        for q in range(4):
                            for kh in range(3):
                                for ci in range(CIB):
                                    lhsT = w_sb[:, ci, q, kh,
                                                co * 128:(co + 1) * 128]
                                    rhs = vt[:, ci, q,
                                             (r0 + kh) * NJ:
                                             (r0 + kh + RB) * NJ]
                                    nc.tensor.matmul(
                                        pss[q][:], lhsT, rhs,
                                        start=(kh == 0 and ci == 0),
                                        stop=(kh == 2 and ci == 1))
                        # ACT evacuates m1 (fast PSUM port); DVE combines,
                        # each op with at most one PSUM operand:
                        #   e1 = m0+c1, o1 = c1-m2, yE = e1+m2, yO = o1-m3
                        ot = opool.tile([128, RB * W], F32, tag="ot",
                                        name=f"ot_{it}_{n}_{co}_{blk}")
                        ot_v = ot[:].rearrange("p (r c) -> p r c", c=W)
                        yE = ot_v[:, :, 0:56:2]
                        yO = ot_v[:, :, 1:56:2]
                        c1 = dpool.tile([128, NMM], F32, tag="c1",
                                        name=f"c1_{it}_{n}_{co}_{blk}")
                        e1 = dpool.tile([128, NMM], F32, tag="e1",
                                        name=f"e1_{it}_{n}_{co}_{blk}")
                        o1 = dpool.tile([128, NMM], F32, tag="o1",
                                        name=f"o1_{it}_{n}_{co}_{blk}")
                        nc.scalar.copy(c1[:], pss[1][:])
                        nc.vector.tensor_add(e1[:], pss[0][:], c1[:])
                        nc.vector.tensor_sub(o1[:], c1[:], pss[2][:])
                        e1v = e1[:].rearrange("p (r j) -> p r j", j=NJ)
                        o1v = o1[:].rearrange("p (r j) -> p r j", j=NJ)
                        ps2v = pss[2][:].rearrange("p (r j) -> p r j", j=NJ)
                        ps3v = pss[3][:].rearrange("p (r j) -> p r j", j=NJ)
                        nc.vector.tensor_add(yE, e1v, ps2v)
                        nc.vector.tensor_sub(yO, o1v, ps3v)

                        # next-image prep slots (after this group's drains
                        # so the psum-free critical path stays short)
                        if nxt is not None:
                            if g == 0:
                                nxt["xt"] = stage_x(it, n + 1,
                                                    (nc.scalar, nc.scalar))
                                nxt["ab"] = new_abcd(it, n + 1)
                                nxt["vt"] = new_vt(it, n + 1)
                            for _ in range(_DEINT_SLOT[g]):
                                ci, letter = _DEINT_ORDER[di]
                                deint_op(nxt["xt"], nxt["ab"], ci, letter)
                                di += 1
                            for _ in range(_V_SLOT[g]):
                                ci, q = _V_ORDER[vi]
                                v_tt(nxt["ab"], nxt["vt"], ci, q)
                                vi += 1

                        nc.sync.dma_start(
                            y_dram[n, co * 128:(co + 1) * 128,
                                   r0:r0 + RB, :],
                            ot[:])

            if loop:
                with tc.For_i(0, loop, 1,
                              hint_engines=(mybir.EngineType.PE,)):
                    emit_iter(0)
            else:
                for it in range(iters):
                    emit_iter(it)

    nc.compile()
    return nc


def _get_compiled(mode):
    if mode not in _COMPILED:
        _COMPILED[mode] = _build(mode)
    return _COMPILED[mode]


def _prep_weights(w_pos, w_neg, mode):
    w_eff = ((w_pos > 0).astype(np.float32)
             - (w_neg > 0).astype(np.float32))          # [CO, CI, 3, 3]
    w0, w1, w2 = w_eff[:, :, :, 0], w_eff[:, :, :, 1], w_eff[:, :, :, 2]
    g = np.stack([w0,
                  0.5 * (w0 + w1 + w2),
                  0.5 * (w0 - w1 + w2),
                  w2], axis=0)                          # [4q, CO, CI, 3kh]
    w_lhsT = np.ascontiguousarray(g.transpose(2, 0, 3, 1))  # [CI, q, kh, CO]
    return w_lhsT.astype(ml_dtypes.bfloat16)            # exact: k/2 values


def kernel(x, w_pos, w_neg):
    mode = MODE
    nc = _get_compiled(mode)
    w_lhsT = _prep_weights(w_pos, w_neg, mode)
    x = np.ascontiguousarray(x, dtype=np.float32)

    in_maps = [
        {"x": x[c * NI:(c + 1) * NI], "w": w_lhsT}
        for c in range(N_CORES)
    ]
    res = run_bass_kernel_spmd(nc, in_maps, list(range(N_CORES)))
    out = np.concatenate([res.results[c]["y"] for c in range(N_CORES)], axis=0)
    return out.astype(np.float32)


# revision 4
# speedup vs baseline: 1.0276x; 1.0276x over previous
"""Trainium2 Bass kernel for nn_Conv2d_77489799955262.

Forward value of the reference:
    y = conv2d(x, (w_pos > 0) - (w_neg > 0))      # ternary weights in {-1, 0, 1}
(the straight-through-estimator terms cancel numerically), NCHW, 3x3, stride 1,
pad 1, x [32, 256, 56, 56] f32, w [256, 256, 3, 3].

Strategy: data-parallel over batch across 8 cores (4 images per core), then
1D Winograd F(2,3) along W inside each core:

    v0 = d[2j]-d[2j+2], v1 = d[2j+1]+d[2j+2],
    v2 = d[2j+2]-d[2j+1], v3 = d[2j+1]-d[2j+3]
    m_q = sum_{kh,ci} wt[q,kh]^T v_q[row+kh]          (PE, PSUM accumulate)
    y_even = m0+m1+m2,  y_odd = m1-m2-m3              (DVE)

HW-measured per-op costs (single-core microbench, exp/micro.py):
    ACT deint copy (fp32 strided -> bf16 packed)  1670 ns
    DVE v-op bf16 step-1 tensor_tensor (2x mode)   872 ns
    DVE v-op fp32 strided (1x fallback)           1725 ns
    DVE drain tensor_tensor fp32 [392]             460 ns
    ACT psum copy [392]                            525 ns
    gpsimd deint copy                             5195 ns (unusable)

Engine budget per image (steady state): PE 31.8us (pacing), DVE ~23us,
ACT ~18us, so the schedule's job is only to keep feeds off the critical
FIFOs:
  - x-in DMA triggers sit at the HEAD of each image's ACT stream (transfer
    lands by ~9us, well before anything reads it); y-out on the sync ring.
  - deint ops for image n+1 slot into groups 3..6 of image n (data already
    landed - an early slot would stall ACT's FIFO on the DMA semaphore and
    push back the c1 psum-evacuation chain, which is what regressed the
    first version of this schedule).
  - v ops for image n+1 slot after the drains of groups 4..7.
"""
import numpy as np
import ml_dtypes

import concourse.bass as bass
import concourse.tile as tile
from concourse import bacc, mybir
from concourse.bass_utils import run_bass_kernel_spmd

MODE = "wino"           # kept for test.py compatibility
PIPELINE = "slots-late"

N_CORES = 8
B, CI, CO, H, W, K = 32, 256, 256, 56, 56, 3
NI = B // N_CORES          # images per core
CIB = CI // 128            # ci blocks
COB = CO // 128            # co blocks
NJ = W // 2                # 28 winograd tiles per row
RB = 14                    # output rows per psum tile
NBLK = H // RB             # 4 row blocks
NMM = RB * NJ              # 392 matmul free dim
VROWS = H + 2              # 58 v rows (0 and 57 are zero pads)
PLANE = H * NJ             # 56*28 deint plane elems

F32 = mybir.dt.float32
BF16 = mybir.dt.bfloat16

_COMPILED = {}

# v_q = lhs op rhs over full [56,28] planes; letters 0..3 = A,B,C,D
_VDEF = [(0, 2, "sub"), (1, 2, "add"), (2, 1, "sub"), (1, 3, "sub")]

# deint plane defs: (letter, out col slice, x col slice (stride 2))
_DEINT = [
    (0, (1, 28), (1, 54)),    # A[j]=x[2j-1], j=1..27; A[0]=0 (pad)
    (1, (0, 28), (0, 55)),    # B[j]=x[2j]
    (2, (0, 28), (1, 56)),    # C[j]=x[2j+1]
    (3, (0, 27), (2, 55)),    # D[j]=x[2j+2], j=0..26; D[27]=0 (pad)
]

# next-image prep slots per group g = co*NBLK + blk. deint order
# (A0,C0,A1,C1,B0,B1,D0,D1), v order (q0c0,q0c1,...,q3c0,q3c1).
_DEINT_SLOT = [0, 0, 0, 2, 2, 2, 2, 0]
_V_SLOT = [0, 0, 0, 0, 2, 2, 2, 2]
_DEINT_ORDER = [(0, 0), (0, 2), (1, 0), (1, 2),
                (0, 1), (1, 1), (0, 3), (1, 3)]   # (ci, letter)
_V_ORDER = [(0, 0), (1, 0), (0, 1), (1, 1),
            (0, 2), (1, 2), (0, 3), (1, 3)]       # (ci, q)


def _build(mode, iters=1, loop=0):
    nc = bacc.Bacc("TRN2", target_bir_lowering=False, debug=False,
                   num_devices=N_CORES)

    x_dram = nc.dram_tensor("x", [NI, CI, H, W], F32, kind="ExternalInput")
    w_dram = nc.dram_tensor("w", [CI, 4, 3, CO], BF16, kind="ExternalInput")
    y_dram = nc.dram_tensor("y", [NI, CO, H, W], F32, kind="ExternalOutput")

    with tile.TileContext(nc) as tc:
        with (
            tc.tile_pool(name="const", bufs=1) as cpool,
            tc.tile_pool(name="xst", bufs=2) as xpool,
            tc.tile_pool(name="abcd", bufs=2) as apool,
            tc.tile_pool(name="vst", bufs=2) as vpool,
            tc.tile_pool(name="drain", bufs=4) as dpool,
            tc.tile_pool(name="outp", bufs=4) as opool,
            tc.tile_pool(name="psum", bufs=8, space="PSUM") as ppool,
        ):
            w_sb = cpool.tile([128, CIB, 4, 3, CO], BF16, tag="w")

            def w_slice(ci, co):
                return (w_sb[:, ci, :, :, co * 128:(co + 1) * 128],
                        w_dram[ci * 128:(ci + 1) * 128, :, :,
                               co * 128:(co + 1) * 128])

            nc.sync.dma_start(*w_slice(0, 0))

            def stage_x(it, n, engs):
                xt = xpool.tile([128, CIB, H * W], F32, tag="x",
                                name=f"x_{it}_{n}")
                for ci in range(CIB):
                    engs[ci].dma_start(
                        xt[:, ci, :].rearrange("p (r c) -> p r c", c=W),
                        x_dram[n, ci * 128:(ci + 1) * 128, :, :])
                return xt

            def new_abcd(it, n):
                ab = apool.tile([128, CIB, 4, PLANE], BF16, tag="ab",
                                name=f"ab_{it}_{n}")
                for ci in range(CIB):
                    av = ab[:, ci, 0, :].rearrange("p (r j) -> p r j", j=NJ)
                    dv = ab[:, ci, 3, :].rearrange("p (r j) -> p r j", j=NJ)
                    nc.gpsimd.memset(av[:, :, 0:1], 0.0)
                    nc.gpsimd.memset(dv[:, :, 27:28], 0.0)
                return ab

            def new_vt(it, n):
                vt = vpool.tile([128, CIB, 4, VROWS * NJ], BF16,
                                tag="v", name=f"v_{it}_{n}")
                vv = vt[:].rearrange("p c q (r j) -> p (c q) r j", j=NJ)
                nc.gpsimd.memset(vv[:, :, 0:1, :], 0.0)
                nc.gpsimd.memset(vv[:, :, 57:58, :], 0.0)
                return vt

            def deint_op(xt, ab, ci, letter):
                _, (o0, o1), (i0, i1) = _DEINT[letter]
                xr = xt[:, ci, :].rearrange("p (r c) -> p r c", c=W)
                pv = (ab[:, ci, letter, :]
                      .rearrange("p (r j) -> p r j", j=NJ))
                nc.scalar.copy(pv[:, :, o0:o1], xr[:, :, i0:i1:2])

            def v_tt(ab, vt, ci, q):
                a, b, op = _VDEF[q]
                out = (vt[:, ci, q, NJ:NJ + H * NJ]
                       .rearrange("p (r j) -> p r j", j=NJ))
                la = ab[:, ci, a, :].rearrange("p (r j) -> p r j", j=NJ)
                lb = ab[:, ci, b, :].rearrange("p (r j) -> p r j", j=NJ)
                f = nc.vector.tensor_add if op == "add" else \
                    nc.vector.tensor_sub
                f(out, la, lb)

            def emit_iter(it):
                # image-0 prologue: x0 split across both HWDGE rings, ACT
                # deint burst, DVE v burst (all dribble in dep order)
                ab = new_abcd(it, 0)
                vt = new_vt(it, 0)
                for ci, co in ((1, 0), (0, 1), (1, 1)):
                    nc.gpsimd.dma_start(*w_slice(ci, co))
                xt = stage_x(it, 0, (nc.scalar, nc.sync))
                for ci, letter in _DEINT_ORDER:
                    deint_op(xt, ab, ci, letter)
                for ci, q in _V_ORDER:
                    v_tt(ab, vt, ci, q)
                for n in range(NI):
                    nxt = {} if n + 1 < NI else None
                    emit_image(it, n, vt, nxt)
                    if nxt is not None:
                        vt = nxt["vt"]

            def emit_image(it, n, vt, nxt):
                if nxt is not None:
                    # x triggers at the head of this image's ACT stream:
                    # the buffer was freed an image ago, so they dispatch
                    # immediately and the transfer lands by ~9us.
                    nxt["xt"] = stage_x(it, n + 1, (nc.scalar, nc.scalar))
                    nxt["ab"] = new_abcd(it, n + 1)
                    nxt["vt"] = new_vt(it, n + 1)
                di = vi = 0
                for co in range(COB):
                    for blk in range(NBLK):
                        g = co * NBLK + blk
                        r0 = blk * RB
                        pss = [ppool.tile([128, NMM], F32, tag="ps",
                                          name=f"ps_{it}_{n}_{co}_{blk}_{q}")
                               for q in range(4)]
                        for q in range(4):
                            for kh in range(3):
                                for ci in range(CIB):
                                    lhsT = w_sb[:, ci, q, kh,
                                                co * 128:(co + 1) * 128]
                                    rhs = vt[:, ci, q,
                                             (r0 + kh) * NJ:
                                             (r0 + kh + RB) * NJ]
                                    nc.tensor.matmul(
                                        pss[q][:], lhsT, rhs,
                                        start=(kh == 0 and ci == 0),
                                        stop=(kh == 2 and ci == 1))
                        #   c1 = m1 (ACT, fast PSUM port)
                        #   e1 = m0+c1, o1 = c1-m2, yE = e1+m2, yO = o1-m3
                        ot = opool.tile([128, RB * W], F32, tag="ot",
                                        name=f"ot_{it}_{n}_{co}_{blk}")
                        ot_v = ot[:].rearrange("p (r c) -> p r c", c=W)
                        yE = ot_v[:, :, 0:56:2]
                        yO = ot_v[:, :, 1:56:2]
                        c1 = dpool.tile([128, NMM], F32, tag="c1",
                                        name=f"c1_{it}_{n}_{co}_{blk}")
                        e1 = dpool.tile([128, NMM], F32, tag="e1",
                                        name=f"e1_{it}_{n}_{co}_{blk}")
                        o1 = dpool.tile([128, NMM], F32, tag="o1",
                                        name=f"o1_{it}_{n}_{co}_{blk}")
                        nc.scalar.copy(c1[:], pss[1][:])
                        nc.vector.tensor_add(e1[:], pss[0][:], c1[:])
                        nc.vector.tensor_sub(o1[:], c1[:], pss[2][:])
                        e1v = e1[:].rearrange("p (r j) -> p r j", j=NJ)
                        o1v = o1[:].rearrange("p (r j) -> p r j", j=NJ)
                        ps2v = pss[2][:].rearrange("p (r j) -> p r j", j=NJ)
                        ps3v = pss[3][:].rearrange("p (r j) -> p r j", j=NJ)
                        nc.vector.tensor_add(yE, e1v, ps2v)
                        nc.vector.tensor_sub(yO, o1v, ps3v)

                        if nxt is not None:
                            for _ in range(_DEINT_SLOT[g]):
                                ci, letter = _DEINT_ORDER[di]
                                deint_op(nxt["xt"], nxt["ab"], ci, letter)
                                di += 1
                            for _ in range(_V_SLOT[g]):
                                ci, q = _V_ORDER[vi]
                                v_tt(nxt["ab"], nxt["vt"], ci, q)
                                vi += 1

                        nc.sync.dma_start(
                            y_dram[n, co * 128:(co + 1) * 128,
                                   r0:r0 + RB, :],
                            ot[:])

            if loop:
                with tc.For_i(0, loop, 1,
                              hint_engines=(mybir.EngineType.PE,)):
                    emit_iter(0)
            else:
                for it in range(iters):
                    emit_iter(it)

    nc.compile()
    return nc


def _get_compiled(mode):
    if mode not in _COMPILED:
        _COMPILED[mode] = _build(mode)
    return _COMPILED[mode]


def _prep_weights(w_pos, w_neg, mode):
    w_eff = ((w_pos > 0).astype(np.float32)
             - (w_neg > 0).astype(np.float32))          # [CO, CI, 3, 3]
    w0, w1, w2 = w_eff[:, :, :, 0], w_eff[:, :, :, 1], w_eff[:, :, :, 2]
    g = np.stack([w0,
                  0.5 * (w0 + w1 + w2),
                  0.5 * (w0 - w1 + w2),
                  w2], axis=0)                          # [4q, CO, CI, 3kh]
    w_lhsT = np.ascontiguousarray(g.transpose(2, 0, 3, 1))  # [CI, q, kh, CO]
    return w_lhsT.astype(ml_dtypes.bfloat16)            # exact: k/2 values


def kernel(x, w_pos, w_neg):
    mode = MODE
    nc = _get_compiled(mode)
    w_lhsT = _prep_weights(w_pos, w_neg, mode)
    x = np.ascontiguousarray(x, dtype=np.float32)

    in_maps = [
        {"x": x[c * NI:(c + 1) * NI], "w": w_lhsT}
        for c in range(N_CORES)
    ]
    res = run_bass_kernel_spmd(nc, in_maps, list(range(N_CORES)))
    out = np.concatenate([res.results[c]["y"] for c in range(N_CORES)], axis=0)
    return out.astype(np.float32)


# revision 5
# speedup vs baseline: 1.1792x; 1.1475x over previous
"""Trainium2 Bass kernel for nn_Conv2d_77489799955262.

Forward value of the reference:
    y = conv2d(x, (w_pos > 0) - (w_neg > 0))      # ternary weights in {-1, 0, 1}
(the straight-through-estimator terms cancel numerically), NCHW, 3x3, stride 1,
pad 1, x [32, 256, 56, 56] f32, w [256, 256, 3, 3].

Strategy: data-parallel over batch across 8 cores (4 images per core), then
1D Winograd F(2,3) along W inside each core:

    v0 = d[2j]-d[2j+2], v1 = d[2j+1]+d[2j+2],
    v2 = d[2j+2]-d[2j+1], v3 = d[2j+1]-d[2j+3]
    m_q = sum_{kh,ci} wt[q,kh]^T v_q[row+kh]          (PE, PSUM accumulate)
    y_even = m0+m1+m2,  y_odd = m1-m2-m3              (DVE)

HW-measured per-op costs (single-core microbench, exp/micro.py):
    ACT deint copy (fp32 strided -> bf16 packed)  1670 ns
    DVE v-op bf16 step-1 tensor_tensor (2x mode)   872 ns
    DVE v-op fp32 strided (1x fallback)           1725 ns
    DVE drain tensor_tensor fp32 [392]             460 ns
    ACT psum copy [392]                            525 ns
    gpsimd deint copy                             5195 ns (unusable)

Engine budget per image (steady state): PE 31.8us (pacing), DVE ~23us,
ACT ~18us, so the schedule's job is only to keep feeds off the critical
FIFOs:
  - x-in DMA triggers sit at the HEAD of each image's ACT stream (transfer
    lands by ~9us, well before anything reads it); y-out on the sync ring.
  - deint ops for image n+1 slot into groups 3..6 of image n (data already
    landed - an early slot would stall ACT's FIFO on the DMA semaphore and
    push back the c1 psum-evacuation chain, which is what regressed the
    first version of this schedule).
  - v ops for image n+1 slot after the drains of groups 4..7.
"""
import numpy as np
import ml_dtypes

import concourse.bass as bass
import concourse.tile as tile
from concourse import bacc, mybir
from concourse.bass_utils import run_bass_kernel_spmd

MODE = "wino"           # kept for test.py compatibility
PIPELINE = "slots-late"

N_CORES = 8
B, CI, CO, H, W, K = 32, 256, 256, 56, 56, 3
NI = B // N_CORES          # images per core
CIB = CI // 128            # ci blocks
COB = CO // 128            # co blocks
NJ = W // 2                # 28 winograd tiles per row
RB = 14                    # output rows per psum tile
NBLK = H // RB             # 4 row blocks
NMM = RB * NJ              # 392 matmul free dim
VROWS = H + 2              # 58 v rows (0 and 57 are zero pads)
PLANE = H * NJ             # 56*28 deint plane elems

F32 = mybir.dt.float32
BF16 = mybir.dt.bfloat16

_COMPILED = {}

# v_q = lhs op rhs over full [56,28] planes; letters 0..3 = A,B,C,D
_VDEF = [(0, 2, "sub"), (1, 2, "add"), (2, 1, "sub"), (1, 3, "sub")]

# deint plane defs: (letter, out col slice, x col slice (stride 2))
_DEINT = [
    (0, (1, 28), (1, 54)),    # A[j]=x[2j-1], j=1..27; A[0]=0 (pad)
    (1, (0, 28), (0, 55)),    # B[j]=x[2j]
    (2, (0, 28), (1, 56)),    # C[j]=x[2j+1]
    (3, (0, 27), (2, 55)),    # D[j]=x[2j+2], j=0..26; D[27]=0 (pad)
]

# next-image prep slots per group g = co*NBLK + blk. deint order
# (A0,C0,A1,C1,B0,B1,D0,D1), v order (q0c0,q0c1,...,q3c0,q3c1).
_DEINT_SLOT = [0, 0, 2, 2, 2, 2, 0, 0]
_V_SLOT = [0, 0, 0, 0, 2, 2, 2, 2]
_DEINT_ORDER = [(0, 0), (0, 2), (1, 0), (1, 2),
                (0, 1), (1, 1), (0, 3), (1, 3)]   # (ci, letter)
_V_ORDER = [(0, 0), (1, 0), (0, 1), (1, 1),
            (0, 2), (1, 2), (0, 3), (1, 3)]       # (ci, q)


def _build(mode, iters=1, loop=0):
    nc = bacc.Bacc("TRN2", target_bir_lowering=False, debug=False,
                   num_devices=N_CORES)

    x_dram = nc.dram_tensor("x", [NI, CI, H, W], F32, kind="ExternalInput")
    w_dram = nc.dram_tensor("w", [CI, 4, 3, CO], BF16, kind="ExternalInput")
    y_dram = nc.dram_tensor("y", [NI, CO, H, W], F32, kind="ExternalOutput")

    with tile.TileContext(nc) as tc:
        with (
            tc.tile_pool(name="const", bufs=1) as cpool,
            tc.tile_pool(name="xst", bufs=2) as xpool,
            tc.tile_pool(name="abcd", bufs=2) as apool,
            tc.tile_pool(name="vst", bufs=2) as vpool,
            tc.tile_pool(name="drain", bufs=3) as dpool,
            tc.tile_pool(name="outp", bufs=3) as opool,
            tc.tile_pool(name="psum", bufs=8, space="PSUM") as ppool,
        ):
            w_sb = cpool.tile([128, CIB, 4, 3, CO], BF16, tag="w")

            def w_slice(ci, co):
                return (w_sb[:, ci, :, :, co * 128:(co + 1) * 128],
                        w_dram[ci * 128:(ci + 1) * 128, :, :,
                               co * 128:(co + 1) * 128])

            nc.sync.dma_start(*w_slice(0, 0))

            def stage_x(it, n, engs):
                xt = xpool.tile([128, CIB, H * W], F32, tag="x",
                                name=f"x_{it}_{n}")
                for ci in range(CIB):
                    engs[ci].dma_start(
                        xt[:, ci, :].rearrange("p (r c) -> p r c", c=W),
                        x_dram[n, ci * 128:(ci + 1) * 128, :, :])
                return xt

            def new_abcd(it, n):
                ab = apool.tile([128, CIB, 4, PLANE], BF16, tag="ab",
                                name=f"ab_{it}_{n}")
                for ci in range(CIB):
                    av = ab[:, ci, 0, :].rearrange("p (r j) -> p r j", j=NJ)
                    dv = ab[:, ci, 3, :].rearrange("p (r j) -> p r j", j=NJ)
                    nc.gpsimd.memset(av[:, :, 0:1], 0.0)
                    nc.gpsimd.memset(dv[:, :, 27:28], 0.0)
                return ab

            def new_vt(it, n):
                vt = vpool.tile([128, CIB, 4, VROWS * NJ], BF16,
                                tag="v", name=f"v_{it}_{n}")
                vv = vt[:].rearrange("p c q (r j) -> p (c q) r j", j=NJ)
                nc.gpsimd.memset(vv[:, :, 0:1, :], 0.0)
                nc.gpsimd.memset(vv[:, :, 57:58, :], 0.0)
                return vt

            def deint_op(xt, ab, ci, letter):
                _, (o0, o1), (i0, i1) = _DEINT[letter]
                xr = xt[:, ci, :].rearrange("p (r c) -> p r c", c=W)
                pv = (ab[:, ci, letter, :]
                      .rearrange("p (r j) -> p r j", j=NJ))
                nc.vector.tensor_copy(pv[:, :, o0:o1], xr[:, :, i0:i1:2])

            def v_tt(ab, vt, ci, q):
                a, b, op = _VDEF[q]
                out = (vt[:, ci, q, NJ:NJ + H * NJ]
                       .rearrange("p (r j) -> p r j", j=NJ))
                la = ab[:, ci, a, :].rearrange("p (r j) -> p r j", j=NJ)
                lb = ab[:, ci, b, :].rearrange("p (r j) -> p r j", j=NJ)
                f = nc.vector.tensor_add if op == "add" else \
                    nc.vector.tensor_sub
                f(out, la, lb)

            def emit_iter(it):
                # image-0 prologue: x0 split across both HWDGE rings, ACT
                # deint burst, DVE v burst (all dribble in dep order)
                ab = new_abcd(it, 0)
                vt = new_vt(it, 0)
                for ci, co in ((1, 0), (0, 1), (1, 1)):
                    nc.gpsimd.dma_start(*w_slice(ci, co))
                xt = stage_x(it, 0, (nc.scalar, nc.sync))
                for ci, letter in _DEINT_ORDER:
                    deint_op(xt, ab, ci, letter)
                for ci, q in _V_ORDER:
                    v_tt(ab, vt, ci, q)
                for n in range(NI):
                    nxt = {} if n + 1 < NI else None
                    emit_image(it, n, vt, nxt)
                    if nxt is not None:
                        vt = nxt["vt"]

            def emit_image(it, n, vt, nxt):
                if nxt is not None:
                    # x triggers at the head of this image's ACT stream:
                    # the buffer was freed an image ago, so they dispatch
                    # immediately and the transfer lands by ~9us.
                    nxt["xt"] = stage_x(it, n + 1, (nc.scalar, nc.scalar))
                    nxt["ab"] = new_abcd(it, n + 1)
                    nxt["vt"] = new_vt(it, n + 1)
                di = vi = 0
                for co in range(COB):
                    for blk in range(NBLK):
                        g = co * NBLK + blk
                        r0 = blk * RB
                        pss = [ppool.tile([128, NMM], F32, tag="ps",
                                          name=f"ps_{it}_{n}_{co}_{blk}_{q}")
                               for q in range(4)]
                        for q in range(4):
                            for kh in range(3):
                                for ci in range(CIB):
                                    lhsT = w_sb[:, ci, q, kh,
                                                co * 128:(co + 1) * 128]
                                    rhs = vt[:, ci, q,
                                             (r0 + kh) * NJ:
                                             (r0 + kh + RB) * NJ]
                                    nc.tensor.matmul(
                                        pss[q][:], lhsT, rhs,
                                        start=(kh == 0 and ci == 0),
                                        stop=(kh == 2 and ci == 1))
                        #   c1 = m1 (ACT, fast PSUM port)
                        #   e1 = m0+c1, o1 = c1-m2, yE = e1+m2, yO = o1-m3
                        ot = opool.tile([128, RB * W], F32, tag="ot",
                                        name=f"ot_{it}_{n}_{co}_{blk}")
                        ot_v = ot[:].rearrange("p (r c) -> p r c", c=W)
                        yE = ot_v[:, :, 0:56:2]
                        yO = ot_v[:, :, 1:56:2]
                        cs = [dpool.tile([128, NMM], F32, tag=f"c{q}",
                                          name=f"c{q}_{it}_{n}_{co}_{blk}")
                              for q in range(4)]
                        for q in range(4):
                            nc.scalar.copy(cs[q][:], pss[q][:])
                        e1 = dpool.tile([128, NMM], F32, tag="e1",
                                        name=f"e1_{it}_{n}_{co}_{blk}")
                        o1 = dpool.tile([128, NMM], F32, tag="o1",
                                        name=f"o1_{it}_{n}_{co}_{blk}")
                        nc.vector.tensor_add(e1[:], cs[0][:], cs[1][:])
                        nc.vector.tensor_sub(o1[:], cs[1][:], cs[2][:])
                        e1v = e1[:].rearrange("p (r j) -> p r j", j=NJ)
                        o1v = o1[:].rearrange("p (r j) -> p r j", j=NJ)
                        c2v = cs[2][:].rearrange("p (r j) -> p r j", j=NJ)
                        c3v = cs[3][:].rearrange("p (r j) -> p r j", j=NJ)
                        nc.vector.tensor_add(yE, e1v, c2v)
                        nc.vector.tensor_sub(yO, o1v, c3v)

                        if nxt is not None:
                            for _ in range(_DEINT_SLOT[g]):
                                ci, letter = _DEINT_ORDER[di]
                                deint_op(nxt["xt"], nxt["ab"], ci, letter)
                                di += 1
                            for _ in range(_V_SLOT[g]):
                                ci, q = _V_ORDER[vi]
                                v_tt(nxt["ab"], nxt["vt"], ci, q)
                                vi += 1

                        nc.sync.dma_start(
                            y_dram[n, co * 128:(co + 1) * 128,
                                   r0:r0 + RB, :],
                            ot[:])

            if loop:
                with tc.For_i(0, loop, 1,
                              hint_engines=(mybir.EngineType.PE,)):
                    emit_iter(0)
            else:
                for it in range(iters):
                    emit_iter(it)

    nc.compile()
    return nc


def _get_compiled(mode):
    if mode not in _COMPILED:
        _COMPILED[mode] = _build(mode)
    return _COMPILED[mode]


def _prep_weights(w_pos, w_neg, mode):
    w_eff = ((w_pos > 0).astype(np.float32)
             - (w_neg > 0).astype(np.float32))          # [CO, CI, 3, 3]
    w0, w1, w2 = w_eff[:, :, :, 0], w_eff[:, :, :, 1], w_eff[:, :, :, 2]
    g = np.stack([w0,
                  0.5 * (w0 + w1 + w2),
                  0.5 * (w0 - w1 + w2),
                  w2], axis=0)                          # [4q, CO, CI, 3kh]
    w_lhsT = np.ascontiguousarray(g.transpose(2, 0, 3, 1))  # [CI, q, kh, CO]
    return w_lhsT.astype(ml_dtypes.bfloat16)            # exact: k/2 values


def kernel(x, w_pos, w_neg):
    mode = MODE
    nc = _get_compiled(mode)
    w_lhsT = _prep_weights(w_pos, w_neg, mode)
    x = np.ascontiguousarray(x, dtype=np.float32)

    in_maps = [
        {"x": x[c * NI:(c + 1) * NI], "w": w_lhsT}
        for c in range(N_CORES)
    ]
    res = run_bass_kernel_spmd(nc, in_maps, list(range(N_CORES)))
    out = np.concatenate([res.results[c]["y"] for c in range(N_CORES)], axis=0)
    return out.astype(np.float32)
